# revision 1
# baseline (speedup 1.0000x reference)
"""ChebGNN encoder (3x ChebConv(K=5) + Linear skip + ReLU) on 8 Trainium2
NeuronCores.

Strategy
--------
* Nodes are sharded over the 8 cores (graph parallel). Each core owns
  TPC dest-tiles of 128 node slots (padded rows; a host-chosen permutation
  bin-packs nodes into tiles so every (core, tile) has the same number of
  128-edge chunks CPT).
* Each Chebyshev layer is evaluated with the Clenshaw recursion
      b4 = Z4;  b_k = Z_k + 2 L b_{k+1} - b_{k+2};  out = Z0 + L b1 - b2
  where Z_k = h @ Wc[k] (Z0 additionally fuses the Linear skip + biases,
  via host-side weight fusion and a K=1 ones x bias matmul).
* The sparse propagation L @ b is computed per dest-tile as
      psum += S_j.T @ X_j      (j = 0..CPT-1)
  with S_j a [128 edge, 128 dest] bf16 block holding 2*w_edge
  (host-precomputed from edge_index) and X_j = dma_gather of the 128
  source rows from the all-gathered state in HBM (512B rows, bf16).
* Between props, the new state shard is DMA'd to a DRAM bounce buffer and
  AllGather'd (shared-output collective) into the gather source buffer.

The full inputs are preprocessed on the host (numpy): edge normalisation,
node permutation, per-core gather index lists and S blocks; the device does
all matmul/prop/activation FLOPs.
"""

import numpy as np
import ml_dtypes

BF16 = ml_dtypes.bfloat16

# ---------------------------------------------------------------- config ---

N = 20000
E = 320000
F_IN = 350
HID = 256
BOT = 64
K = 5
NCORES = 8
CHUNK = 128
TPC = 20                    # dest tiles per core
NROWS = TPC * CHUNK         # padded rows per core (2560)
KPAD_IN = 384               # F_IN padded to 3*128
F2PAD = 128                 # layer-2 output width padded (BOT=64 real)


def _layer_dims(li):
    # (kpad = padded contraction dim, f_out = padded output width)
    return ((KPAD_IN, HID), (HID, HID), (HID, F2PAD))[li]


# ------------------------------------------------------ host preprocessing ---


def _edge_norm_host(edge_index):
    row = edge_index[0].astype(np.int64)
    col = edge_index[1].astype(np.int64)
    mask = row != col
    deg = np.bincount(row[mask], minlength=N).astype(np.float32)
    with np.errstate(divide="ignore"):
        dinv = np.where(deg > 0, 1.0 / np.sqrt(np.maximum(deg, 1e-12)), 0.0).astype(
            np.float32
        )
    w = (-dinv[row] * dinv[col]).astype(np.float32)
    w = np.where(mask, w, 0.0)
    return row, col, w, mask


def _build_all(edge_index):
    """Node permutation + per-core padded edge structure.

    Returns (meta, cores): meta has gslot [N] (node -> padded global slot),
    slot_node [NCORES*NROWS] (slot -> node or -1), cpt; cores[c] has
    idx/sval/dloc arrays of length epad = TPC*cpt*128.
    """
    row, col, w, mask = _edge_norm_host(edge_index)
    indeg = np.bincount(col[mask], minlength=N).astype(np.int64)

    nloc = N // NCORES
    order = np.argsort(-indeg, kind="stable")
    node_core = np.empty(N, dtype=np.int64)
    rounds = np.arange(N) // NCORES
    fwd = np.arange(N) % NCORES
    snake = np.where(rounds % 2 == 0, fwd, NCORES - 1 - fwd)
    node_core[order] = snake

    tile_of_node = np.empty(N, dtype=np.int64)
    slot_of_node = np.empty(N, dtype=np.int64)
    for c in range(NCORES):
        nodes_c = np.flatnonzero(node_core == c)
        assert len(nodes_c) == nloc
        heavy_first = nodes_c[np.argsort(-indeg[nodes_c], kind="stable")]
        loads = np.zeros(TPC, dtype=np.int64)
        counts = np.zeros(TPC, dtype=np.int64)
        for nd in heavy_first:
            cand = np.flatnonzero(counts < CHUNK)
            t = cand[np.argmin(loads[cand])]
            tile_of_node[nd] = t
            slot_of_node[nd] = counts[t]
            counts[t] += 1
            loads[t] += indeg[nd]

    gslot = node_core * NROWS + tile_of_node * CHUNK + slot_of_node
    slot_node = np.full(NCORES * NROWS, -1, dtype=np.int64)
    slot_node[gslot] = np.arange(N)

    er, ec, ew = row[mask], col[mask], w[mask]
    dest_core = node_core[ec]
    dest_tile = tile_of_node[ec]
    dest_slot = slot_of_node[ec]
    src_gslot = gslot[er]

    counts = np.zeros((NCORES, TPC), dtype=np.int64)
    np.add.at(counts, (dest_core, dest_tile), 1)
    cpt = int(np.ceil(counts.max() / CHUNK))
    epad = TPC * cpt * CHUNK

    cores = []
    for c in range(NCORES):
        m = dest_core == c
        t_arr, s_arr = dest_tile[m], dest_slot[m]
        src_arr, w_arr = src_gslot[m], ew[m]
        o = np.argsort(t_arr, kind="stable")
        t_arr, s_arr, src_arr, w_arr = t_arr[o], s_arr[o], src_arr[o], w_arr[o]
        idx = np.zeros(epad, dtype=np.int16)
        sval = np.zeros(epad, dtype=np.float32)
        dloc = np.zeros(epad, dtype=np.int16)
        starts = np.searchsorted(t_arr, np.arange(TPC))
        ends = np.searchsorted(t_arr, np.arange(TPC) + 1)
        for t in range(TPC):
            a, b = starts[t], ends[t]
            base = t * cpt * CHUNK
            idx[base : base + b - a] = src_arr[a:b]
            sval[base : base + b - a] = 2.0 * w_arr[a:b]
            dloc[base : base + b - a] = s_arr[a:b]
        cores.append((idx, sval, dloc))

    meta = dict(gslot=gslot, slot_node=slot_node, cpt=cpt, epad=epad)
    return meta, cores


def _pack_idx(idx, piece):
    """dma_gather index layout: flat gathered position i within a piece reads
    idx_sb[i % 16, i // 16] (column-major wrap over 16 partitions), pieces
    side by side along the free dim, replicated to 128 partitions."""
    epad = idx.shape[0]
    blocks = [
        np.ascontiguousarray(idx[g * piece : (g + 1) * piece].reshape(piece // 16, 16).T)
        for g in range(epad // piece)
    ]
    arr = np.concatenate(blocks, axis=1)
    return np.ascontiguousarray(np.tile(arr, (8, 1))).astype(np.int16)


def _build_sblocks(sval, dloc, epad):
    s = np.zeros((CHUNK, epad), dtype=np.float32)
    flat = np.arange(epad)
    j = flat // CHUNK
    kk = flat % CHUNK
    s[kk, j * CHUNK + dloc.astype(np.int64)] = sval
    return s.astype(BF16)


def _fuse_weights(inputs):
    """Per layer: (wf [kpad, f_out] = Wc[0]+Wl (+0 pad), wrest [4, kpad, f_out],
    bias [1, f_out] = bc+bl), all bf16."""
    out = []
    for li in range(3):
        kpad, f_out = _layer_dims(li)
        Wc = np.asarray(inputs[f"Wc{li}"], np.float32)
        Wl = np.asarray(inputs[f"Wl{li}"], np.float32)
        bc = np.asarray(inputs[f"bc{li}"], np.float32)
        bl = np.asarray(inputs[f"bl{li}"], np.float32)
        f_in, f_real = Wc.shape[1], Wc.shape[2]
        wf = np.zeros((kpad, f_out), np.float32)
        wf[:f_in, :f_real] = Wc[0] + Wl
        wrest = np.zeros((K - 1, kpad, f_out), np.float32)
        for k in range(1, K):
            wrest[k - 1, :f_in, :f_real] = Wc[k]
        bias = np.zeros((1, f_out), np.float32)
        bias[0, :f_real] = bc + bl
        out.append((wf.astype(BF16), wrest.astype(BF16), bias.astype(BF16)))
    return out


# ------------------------------------------------------------- device side ---


NQUEUES = 4


def build_bass(cpt, reps=1):
    import concourse.bacc as bacc
    import concourse.bass as bass
    import concourse.mybir as mybir
    import concourse.tile as tile
    from concourse import library_config

    dt = mybir.dt
    epad = TPC * cpt * CHUNK

    nc = bacc.Bacc(
        "TRN2",
        target_bir_lowering=False,
        debug=False,
        num_devices=NCORES,
        num_swdge_queues=NQUEUES,
    )

    # ---- I/O ----
    xT_d = nc.dram_tensor("xT", [KPAD_IN, NROWS], dt.bfloat16, kind="ExternalInput")
    idx_d = nc.dram_tensor("idx", [128, epad // 16], dt.int16, kind="ExternalInput")
    sblk_d = nc.dram_tensor("sblk", [128, epad], dt.bfloat16, kind="ExternalInput")
    ident_d = nc.dram_tensor("ident", [128, 128], dt.bfloat16, kind="ExternalInput")
    w_d = []
    for li in range(3):
        kpad, f_out = _layer_dims(li)
        w_d.append(
            (
                nc.dram_tensor(f"wf{li}", [kpad, f_out], dt.bfloat16, kind="ExternalInput"),
                nc.dram_tensor(
                    f"wr{li}", [K - 1, kpad, f_out], dt.bfloat16, kind="ExternalInput"
                ),
                nc.dram_tensor(f"bias{li}", [1, f_out], dt.bfloat16, kind="ExternalInput"),
            )
        )
    out_d = nc.dram_tensor("out_sh", [NROWS, F2PAD], dt.float32, kind="ExternalOutput")

    # internal DRAM for the state exchange
    bounce_a = nc.dram_tensor("bounce_a", [NROWS, HID], dt.bfloat16, kind="Internal")
    bfull_a = nc.dram_tensor(
        "bfull_a", [NCORES * NROWS, HID], dt.bfloat16, kind="Internal", addr_space="Shared"
    )
    bounce_b = nc.dram_tensor("bounce_b", [NROWS, F2PAD], dt.bfloat16, kind="Internal")
    bfull_b = nc.dram_tensor(
        "bfull_b",
        [NCORES * NROWS, F2PAD],
        dt.bfloat16,
        kind="Internal",
        addr_space="Shared",
    )

    groups = [list(range(NCORES))]

    with tile.TileContext(nc) as tc:
        with (
            tc.tile_pool(name="const", bufs=1) as cpool,
            tc.tile_pool(name="hT", bufs=2) as hpool,
            tc.tile_pool(name="bstate", bufs=3) as bpool,
            tc.tile_pool(name="xbuf", bufs=2) as xpool,
            tc.tile_pool(name="small", bufs=4) as spool,
            tc.tile_pool(name="zpsum", bufs=2, space="PSUM") as zpool,
            tc.tile_pool(name="rpsum", bufs=2, space="PSUM") as rpool,
            tc.tile_pool(name="tpsum", bufs=2, space="PSUM") as tpool,
        ):
            nc.gpsimd.load_library(library_config.mlp)

            # ---- resident loads ----
            sblk_sb = cpool.tile([128, epad], dt.bfloat16, tag="sblk")
            nc.sync.dma_start(sblk_sb[:], sblk_d.ap())
            idx_sb = cpool.tile([128, epad // 16], dt.int16, tag="idx")
            nc.sync.dma_start(idx_sb[:], idx_d.ap())
            ident_sb = cpool.tile([128, 128], dt.bfloat16, tag="ident")
            nc.sync.dma_start(ident_sb[:], ident_d.ap())
            ones_sb = cpool.tile([1, 128], dt.bfloat16, tag="ones")
            nc.vector.memset(ones_sb[:], 1.0)

            w_sb = []
            for li in range(3):
                kpad, f_out = _layer_dims(li)
                kc = kpad // 128
                wf_sb = cpool.tile([128, kc, f_out], dt.bfloat16, tag=f"wf{li}")
                nc.sync.dma_start(
                    wf_sb[:], w_d[li][0].ap().rearrange("(c p) f -> p c f", p=128)
                )
                wr_sb = cpool.tile([128, K - 1, kc, f_out], dt.bfloat16, tag=f"wr{li}")
                nc.sync.dma_start(
                    wr_sb[:], w_d[li][1].ap().rearrange("k (c p) f -> p k c f", p=128)
                )
                bias_sb = cpool.tile([1, f_out], dt.bfloat16, tag=f"bias{li}")
                nc.sync.dma_start(bias_sb[:], w_d[li][2].ap())
                w_sb.append((wf_sb, wr_sb, bias_sb))

            hT0 = cpool.tile([128, KPAD_IN // 128, NROWS], dt.bfloat16, tag="hT0")
            nc.sync.dma_start(hT0[:], xT_d.ap().rearrange("(c p) n -> p c n", p=128))

            def dense_tile(zp, hT_in, li, widx, t, with_bias):
                """psum[128 nodes, f_out] = h_tile @ W  (+ ones x bias)."""
                kpad, f_out = _layer_dims(li)
                kc = kpad // 128
                wf_sb, wr_sb, bias_sb = w_sb[li]
                zv = zp[:, :f_out]
                for c in range(kc):
                    lhsT = hT_in[:, c, t * 128 : (t + 1) * 128]
                    rhs = wf_sb[:, c, :] if widx == 0 else wr_sb[:, widx - 1, c, :]
                    nc.tensor.matmul(
                        zv,
                        lhsT,
                        rhs,
                        start=(c == 0),
                        stop=(c == kc - 1 and not with_bias),
                    )
                if with_bias:
                    nc.tensor.matmul(
                        zv, ones_sb[:1, :], bias_sb[:1, :], start=False, stop=True
                    )

            def run_layer(li, hT_in, hT_out, bfull, bounce):
                kpad, f_out = _layer_dims(li)
                fo = f_out

                # --- b4 = Z4, straight to bounce + SBUF state ---
                b4 = bpool.tile([128, TPC, HID], dt.bfloat16, tag="bst")
                for t in range(TPC):
                    zp = zpool.tile([128, HID], dt.float32, tag="z")
                    dense_tile(zp, hT_in, li, 4, t, False)
                    nc.vector.tensor_copy(b4[:, t, :fo], zp[:, :fo])
                    nc.sync.dma_start(
                        bounce.ap()[t * 128 : (t + 1) * 128, :], b4[:, t, :fo]
                    )
                nc.gpsimd.collective_compute(
                    "AllGather",
                    mybir.AluOpType.bypass,
                    replica_groups=groups,
                    ins=[bounce.ap().opt()],
                    outs=[bfull.ap().opt()],
                )

                b_prev2 = None  # b_{k+2}
                b_prev1 = b4  # b_{k+1} (already in bfull)
                for kth in (3, 2, 1, 0):
                    is_final = kth == 0
                    b_new = (
                        None
                        if is_final
                        else bpool.tile([128, TPC, HID], dt.bfloat16, tag="bst")
                    )
                    for t in range(TPC):
                        X = xpool.tile([128, cpt, fo], dt.bfloat16, tag="X")
                        nc.gpsimd.dma_gather(
                            X[:],
                            bfull.ap(),
                            idx_sb[:, t * 128 : (t + 1) * 128],
                            cpt * CHUNK,
                            cpt * CHUNK,
                            fo,
                            single_packet=False,
                            queue_num=t % NQUEUES,
                        )
                        rp = rpool.tile([128, HID], dt.float32, tag="r")
                        for j in range(cpt):
                            e0 = (t * cpt + j) * CHUNK
                            nc.tensor.matmul(
                                rp[:, :fo],
                                sblk_sb[:, e0 : e0 + CHUNK],
                                X[:, j, :],
                                start=(j == 0),
                                stop=(j == cpt - 1),
                            )
                        zp = zpool.tile([128, HID], dt.float32, tag="z")
                        dense_tile(zp, hT_in, li, kth, t, is_final)
                        z_sb = spool.tile([128, HID], dt.bfloat16, tag="ztmp")
                        nc.vector.tensor_copy(z_sb[:, :fo], zp[:, :fo])
                        if kth == 3:
                            nc.vector.tensor_add(
                                b_new[:, t, :fo], z_sb[:, :fo], rp[:, :fo]
                            )
                        elif not is_final:
                            tmp = spool.tile([128, HID], dt.bfloat16, tag="ttmp")
                            nc.vector.tensor_add(tmp[:, :fo], z_sb[:, :fo], rp[:, :fo])
                            nc.vector.tensor_sub(
                                b_new[:, t, :fo], tmp[:, :fo], b_prev2[:, t, :fo]
                            )
                        else:
                            # out = relu(Z0L + 0.5*P2 - b2)
                            a1 = spool.tile([128, HID], dt.float32, tag="a1")
                            nc.vector.tensor_scalar_mul(a1[:, :fo], rp[:, :fo], 0.5)
                            a2 = spool.tile([128, HID], dt.bfloat16, tag="ttmp")
                            nc.vector.tensor_sub(
                                a2[:, :fo], z_sb[:, :fo], b_prev2[:, t, :fo]
                            )
                            if li < 2:
                                h = spool.tile([128, HID], dt.bfloat16, tag="h")
                                nc.vector.tensor_add(h[:, :fo], a1[:, :fo], a2[:, :fo])
                                nc.vector.tensor_relu(h[:, :fo], h[:, :fo])
                                # transpose into hT_out
                                for c2 in range(fo // 128):
                                    tp = tpool.tile([128, 128], dt.bfloat16, tag="tp")
                                    nc.tensor.transpose(
                                        tp[:],
                                        h[:, c2 * 128 : (c2 + 1) * 128],
                                        ident_sb[:],
                                    )
                                    nc.vector.tensor_copy(
                                        hT_out[:, c2, t * 128 : (t + 1) * 128], tp[:]
                                    )
                            else:
                                hf = spool.tile([128, F2PAD], dt.float32, tag="hf")
                                nc.vector.tensor_add(hf[:], a1[:, :fo], a2[:, :fo])
                                nc.vector.tensor_relu(hf[:], hf[:])
                                nc.sync.dma_start(
                                    out_d.ap()[t * 128 : (t + 1) * 128, :], hf[:]
                                )
                        if not is_final:
                            nc.sync.dma_start(
                                bounce.ap()[t * 128 : (t + 1) * 128, :],
                                b_new[:, t, :fo],
                            )
                    if not is_final:
                        nc.gpsimd.collective_compute(
                            "AllGather",
                            mybir.AluOpType.bypass,
                            replica_groups=groups,
                            ins=[bounce.ap().opt()],
                            outs=[bfull.ap().opt()],
                        )
                        b_prev2 = b_prev1
                        b_prev1 = b_new

            for _ in range(reps):
                hT1 = hpool.tile([128, HID // 128, NROWS], dt.bfloat16, tag="hTn")
                run_layer(0, hT0, hT1, bfull_a, bounce_a)
                hT2 = hpool.tile([128, HID // 128, NROWS], dt.bfloat16, tag="hTn")
                run_layer(1, hT1, hT2, bfull_a, bounce_a)
                run_layer(2, hT2, None, bfull_b, bounce_b)

    nc.compile()
    return nc


# ----------------------------------------------------------------- runner ---

_CACHE = {}


def _get_nc(cpt, reps=1):
    key = (cpt, reps)
    if key not in _CACHE:
        _CACHE[key] = build_bass(cpt, reps)
    return _CACHE[key]


def make_in_maps(inputs):
    x = np.asarray(inputs["x"], np.float32)
    edge_index = np.asarray(inputs["edge_index"])
    meta, cores = _build_all(edge_index)
    gslot, slot_node = meta["gslot"], meta["slot_node"]
    cpt, epad = meta["cpt"], meta["epad"]

    weights = _fuse_weights(inputs)
    ident = np.eye(128, dtype=BF16)

    x_slot = np.zeros((NCORES * NROWS, KPAD_IN), np.float32)
    x_slot[gslot, :F_IN] = x

    in_maps = []
    for c in range(NCORES):
        idx, sval, dloc = cores[c]
        m = {
            "xT": np.ascontiguousarray(
                x_slot[c * NROWS : (c + 1) * NROWS].T
            ).astype(BF16),
            "idx": _pack_idx(idx, cpt * CHUNK),
            "sblk": _build_sblocks(sval, dloc, epad),
            "ident": ident,
        }
        for li in range(3):
            wf, wrest, bias = weights[li]
            m[f"wf{li}"] = wf
            m[f"wr{li}"] = wrest
            m[f"bias{li}"] = bias
        in_maps.append(m)
    return in_maps, meta


def assemble_output(results, meta):
    slot_node = meta["slot_node"]
    out_slot = np.concatenate([r["out_sh"] for r in results], axis=0)
    out = np.zeros((N, BOT), np.float32)
    valid = slot_node >= 0
    out[slot_node[valid]] = out_slot[valid][:, :BOT]
    return out


def kernel(**inputs):
    from concourse import bass_utils

    in_maps, meta = make_in_maps(inputs)
    nc = _get_nc(meta["cpt"])
    res = bass_utils.run_bass_kernel_spmd(nc, in_maps, core_ids=list(range(NCORES)))
    return assemble_output(res.results, meta)



# revision 16
# speedup vs baseline: 3.9645x; 3.9645x over previous
"""ChebGNN encoder (3x ChebConv(K=5) + Linear skip + ReLU) on 8 Trainium2
NeuronCores.

Strategy
--------
* Nodes are sharded over the 8 cores (graph parallel). Each core owns
  TPC dest-tiles of 128 node slots (padded rows; a host-chosen permutation
  bin-packs nodes into tiles so every (core, tile) has the same number of
  128-edge chunks CPT).
* Each Chebyshev layer is evaluated with the Clenshaw recursion
      b4 = Z4;  b_k = Z_k + 2 L b_{k+1} - b_{k+2};  out = Z0 + L b1 - b2
  where Z_k = h @ Wc[k] (Z0 additionally fuses the Linear skip + biases,
  via host-side weight fusion and a K=1 ones x bias matmul).
* The sparse propagation L @ b is computed per dest-tile as
      psum += S_j.T @ X_j      (j = 0..CPT-1)
  with S_j a [128 edge, 128 dest] bf16 block holding 2*w_edge
  (host-precomputed from edge_index) and X_j = dma_gather of the 128
  source rows from the all-gathered state in HBM (512B rows, bf16).
* Between props, the new state shard is DMA'd to a DRAM bounce buffer and
  AllGather'd (shared-output collective) into the gather source buffer.

The full inputs are preprocessed on the host (numpy): edge normalisation,
node permutation, per-core gather index lists and S blocks; the device does
all matmul/prop/activation FLOPs.
"""

import numpy as np
import ml_dtypes

BF16 = ml_dtypes.bfloat16

# ---------------------------------------------------------------- config ---

N = 20000
E = 320000
F_IN = 350
HID = 256
BOT = 64
K = 5
NCORES = 8
CHUNK = 128
TPC = 20                    # dest tiles per core
NROWS = TPC * CHUNK         # padded rows per core (2560)
KPAD_IN = 384               # F_IN padded to 3*128
F2PAD = 128                 # layer-2 output width padded (BOT=64 real)


def _layer_dims(li):
    # (kpad = padded contraction dim, f_out = padded output width)
    return ((KPAD_IN, HID), (HID, HID), (HID, F2PAD))[li]


# ------------------------------------------------------ host preprocessing ---


def _edge_norm_host(edge_index):
    row = edge_index[0].astype(np.int64)
    col = edge_index[1].astype(np.int64)
    mask = row != col
    deg = np.bincount(row[mask], minlength=N).astype(np.float32)
    with np.errstate(divide="ignore"):
        dinv = np.where(deg > 0, 1.0 / np.sqrt(np.maximum(deg, 1e-12)), 0.0).astype(
            np.float32
        )
    w = (-dinv[row] * dinv[col]).astype(np.float32)
    w = np.where(mask, w, 0.0)
    return row, col, w, mask


def _build_all(edge_index):
    """Node permutation + per-core padded edge structure.

    Returns (meta, cores): meta has gslot [N] (node -> padded global slot),
    slot_node [NCORES*NROWS] (slot -> node or -1), cpt; cores[c] has
    idx/sval/dloc arrays of length epad = TPC*cpt*128.
    """
    row, col, w, mask = _edge_norm_host(edge_index)
    indeg = np.bincount(col[mask], minlength=N).astype(np.int64)

    nloc = N // NCORES
    order = np.argsort(-indeg, kind="stable")
    node_core = np.empty(N, dtype=np.int64)
    rounds = np.arange(N) // NCORES
    fwd = np.arange(N) % NCORES
    snake = np.where(rounds % 2 == 0, fwd, NCORES - 1 - fwd)
    node_core[order] = snake

    tile_of_node = np.empty(N, dtype=np.int64)
    slot_of_node = np.empty(N, dtype=np.int64)
    for c in range(NCORES):
        nodes_c = np.flatnonzero(node_core == c)
        assert len(nodes_c) == nloc
        heavy_first = nodes_c[np.argsort(-indeg[nodes_c], kind="stable")]
        loads = np.zeros(TPC, dtype=np.int64)
        counts = np.zeros(TPC, dtype=np.int64)
        for nd in heavy_first:
            cand = np.flatnonzero(counts < CHUNK)
            t = cand[np.argmin(loads[cand])]
            tile_of_node[nd] = t
            slot_of_node[nd] = counts[t]
            counts[t] += 1
            loads[t] += indeg[nd]

    gslot = node_core * NROWS + tile_of_node * CHUNK + slot_of_node
    slot_node = np.full(NCORES * NROWS, -1, dtype=np.int64)
    slot_node[gslot] = np.arange(N)

    er, ec, ew = row[mask], col[mask], w[mask]
    dest_core = node_core[ec]
    dest_tile = tile_of_node[ec]
    dest_slot = slot_of_node[ec]
    src_gslot = gslot[er]

    counts = np.zeros((NCORES, TPC), dtype=np.int64)
    np.add.at(counts, (dest_core, dest_tile), 1)
    cpt = int(np.ceil(counts.max() / CHUNK))
    epad = TPC * cpt * CHUNK

    cores = []
    for c in range(NCORES):
        m = dest_core == c
        t_arr, s_arr = dest_tile[m], dest_slot[m]
        src_arr, w_arr = src_gslot[m], ew[m]
        o = np.argsort(t_arr, kind="stable")
        t_arr, s_arr, src_arr, w_arr = t_arr[o], s_arr[o], src_arr[o], w_arr[o]
        idx = np.zeros(epad, dtype=np.int16)
        sval = np.zeros(epad, dtype=np.float32)
        dloc = np.zeros(epad, dtype=np.int16)
        starts = np.searchsorted(t_arr, np.arange(TPC))
        ends = np.searchsorted(t_arr, np.arange(TPC) + 1)
        for t in range(TPC):
            a, b = starts[t], ends[t]
            base = t * cpt * CHUNK
            idx[base : base + b - a] = src_arr[a:b]
            sval[base : base + b - a] = 2.0 * w_arr[a:b]
            dloc[base : base + b - a] = s_arr[a:b]
        cores.append((idx, sval, dloc))

    meta = dict(gslot=gslot, slot_node=slot_node, cpt=cpt, epad=epad)
    return meta, cores


def _pack_idx(idx, piece):
    """dma_gather index layout: flat gathered position i within a piece reads
    idx_sb[i % 16, i // 16] (column-major wrap over 16 partitions), pieces
    side by side along the free dim, replicated to 128 partitions."""
    epad = idx.shape[0]
    blocks = [
        np.ascontiguousarray(idx[g * piece : (g + 1) * piece].reshape(piece // 16, 16).T)
        for g in range(epad // piece)
    ]
    arr = np.concatenate(blocks, axis=1)
    return np.ascontiguousarray(np.tile(arr, (8, 1))).astype(np.int16)


def _build_sblocks(sval, dloc, epad):
    s = np.zeros((CHUNK, epad), dtype=np.float32)
    flat = np.arange(epad)
    j = flat // CHUNK
    kk = flat % CHUNK
    s[kk, j * CHUNK + dloc.astype(np.int64)] = sval
    return s.astype(BF16)


def _fuse_weights(inputs):
    """Per layer: (wf [kpad, f_out] = Wc[0]+Wl (+0 pad), wrest [4, kpad, f_out],
    bias [1, f_out] = bc+bl), all bf16."""
    out = []
    for li in range(3):
        kpad, f_out = _layer_dims(li)
        Wc = np.asarray(inputs[f"Wc{li}"], np.float32)
        Wl = np.asarray(inputs[f"Wl{li}"], np.float32)
        bc = np.asarray(inputs[f"bc{li}"], np.float32)
        bl = np.asarray(inputs[f"bl{li}"], np.float32)
        f_in, f_real = Wc.shape[1], Wc.shape[2]
        wf = np.zeros((kpad, f_out), np.float32)
        wf[:f_in, :f_real] = Wc[0] + Wl
        wrest = np.zeros((K - 1, kpad, f_out), np.float32)
        for k in range(1, K):
            wrest[k - 1, :f_in, :f_real] = Wc[k]
        bias = np.zeros((1, f_out), np.float32)
        bias[0, :f_real] = bc + bl
        out.append((wf.astype(BF16), wrest.astype(BF16), bias.astype(BF16)))
    return out


# ------------------------------------------------------------- device side ---


NQUEUES = 4
GGRP = 2                    # dest tiles per dma_gather call


def build_bass(cpt, reps=1, no_collective=False, tiny_collective=False, no_gather=False):
    import concourse.bacc as bacc
    import concourse.bass as bass
    import concourse.mybir as mybir
    import concourse.tile as tile
    from concourse import library_config

    dt = mybir.dt
    epad = TPC * cpt * CHUNK

    nc = bacc.Bacc(
        "TRN2",
        target_bir_lowering=False,
        debug=False,
        num_devices=NCORES,
        num_swdge_queues=NQUEUES,
    )

    # ---- I/O ----
    xT_d = nc.dram_tensor("xT", [KPAD_IN, NROWS], dt.bfloat16, kind="ExternalInput")
    idx_d = nc.dram_tensor("idx", [128, epad // 16], dt.int16, kind="ExternalInput")
    sblk_d = nc.dram_tensor("sblk", [128, epad], dt.bfloat16, kind="ExternalInput")
    ident_d = nc.dram_tensor("ident", [128, 128], dt.bfloat16, kind="ExternalInput")
    w_d = []
    for li in range(3):
        kpad, f_out = _layer_dims(li)
        w_d.append(
            (
                nc.dram_tensor(f"wf{li}", [kpad, f_out], dt.bfloat16, kind="ExternalInput"),
                nc.dram_tensor(
                    f"wr{li}", [K - 1, kpad, f_out], dt.bfloat16, kind="ExternalInput"
                ),
                nc.dram_tensor(f"bias{li}", [1, f_out], dt.bfloat16, kind="ExternalInput"),
            )
        )
    out_d = nc.dram_tensor("out_sh", [NROWS, F2PAD], dt.float32, kind="ExternalOutput")

    # internal DRAM for the state exchange
    bounce_a = nc.dram_tensor("bounce_a", [NROWS, HID], dt.bfloat16, kind="Internal")
    bfull_a = nc.dram_tensor(
        "bfull_a", [NCORES * NROWS, HID], dt.bfloat16, kind="Internal", addr_space="Shared"
    )
    bounce_b = nc.dram_tensor("bounce_b", [NROWS, F2PAD], dt.bfloat16, kind="Internal")
    bfull_b = nc.dram_tensor(
        "bfull_b",
        [NCORES * NROWS, F2PAD],
        dt.bfloat16,
        kind="Internal",
        addr_space="Shared",
    )

    groups = [list(range(NCORES))]

    with tile.TileContext(nc) as tc:
        with (
            tc.tile_pool(name="const", bufs=1) as cpool,
            tc.tile_pool(name="hT", bufs=2) as hpool,
            tc.tile_pool(name="bstate", bufs=3) as bpool,
            tc.tile_pool(name="xbuf", bufs=2) as xpool,
            tc.tile_pool(name="small", bufs=3) as spool,
            tc.tile_pool(name="zpsum", bufs=2, space="PSUM") as zpool,
            tc.tile_pool(name="rpsum", bufs=2, space="PSUM") as rpool,
            tc.tile_pool(name="tpsum", bufs=2, space="PSUM") as tpool,
        ):
            nc.gpsimd.load_library(library_config.mlp)

            # ---- resident loads ----
            sblk_sb = cpool.tile([128, epad], dt.bfloat16, tag="sblk")
            nc.sync.dma_start(sblk_sb[:], sblk_d.ap())
            idx_sb = cpool.tile([128, epad // 16], dt.int16, tag="idx")
            nc.sync.dma_start(idx_sb[:], idx_d.ap())
            ident_sb = cpool.tile([128, 128], dt.bfloat16, tag="ident")
            nc.sync.dma_start(ident_sb[:], ident_d.ap())
            ones_sb = cpool.tile([1, 128], dt.bfloat16, tag="ones")
            nc.vector.memset(ones_sb[:], 1.0)

            w_sb = []
            for li in range(3):
                kpad, f_out = _layer_dims(li)
                kc = kpad // 128
                wf_sb = cpool.tile([128, kc, f_out], dt.bfloat16, tag=f"wf{li}")
                nc.sync.dma_start(
                    wf_sb[:], w_d[li][0].ap().rearrange("(c p) f -> p c f", p=128)
                )
                wr_sb = cpool.tile([128, K - 1, kc, f_out], dt.bfloat16, tag=f"wr{li}")
                nc.sync.dma_start(
                    wr_sb[:], w_d[li][1].ap().rearrange("k (c p) f -> p k c f", p=128)
                )
                bias_sb = cpool.tile([1, f_out], dt.bfloat16, tag=f"bias{li}")
                nc.sync.dma_start(bias_sb[:], w_d[li][2].ap())
                w_sb.append((wf_sb, wr_sb, bias_sb))

            hT0 = cpool.tile([128, KPAD_IN // 128, NROWS], dt.bfloat16, tag="hT0")
            nc.sync.dma_start(hT0[:], xT_d.ap().rearrange("(c p) n -> p c n", p=128))

            gctr = [0]

            def gather_q():
                q = gctr[0] % NQUEUES
                gctr[0] += 1
                return q

            def do_exchange(bounce, bfull):
                if no_collective:
                    return
                if tiny_collective:
                    ins_ap = bounce.ap()[0:128, :].opt()
                    outs_ap = bfull.ap()[0 : 128 * NCORES, :].opt()
                else:
                    ins_ap = bounce.ap().opt()
                    outs_ap = bfull.ap().opt()
                nc.gpsimd.collective_compute(
                    "AllGather",
                    mybir.AluOpType.bypass,
                    replica_groups=groups,
                    ins=[ins_ap],
                    outs=[outs_ap],
                )

            def dense_tile(zp, hT_in, li, widx, t, with_bias):
                """psum[128 nodes, f_out] = h_tile @ W  (+ ones x bias)."""
                kpad, f_out = _layer_dims(li)
                kc = kpad // 128
                wf_sb, wr_sb, bias_sb = w_sb[li]
                zv = zp[:, :f_out]
                for c in range(kc):
                    lhsT = hT_in[:, c, t * 128 : (t + 1) * 128]
                    rhs = wf_sb[:, c, :] if widx == 0 else wr_sb[:, widx - 1, c, :]
                    nc.tensor.matmul(
                        zv,
                        lhsT,
                        rhs,
                        start=(c == 0),
                        stop=(c == kc - 1 and not with_bias),
                    )
                if with_bias:
                    nc.tensor.matmul(
                        zv, ones_sb[:1, :], bias_sb[:1, :], start=False, stop=True
                    )

            def run_layer(li, hT_in, hT_out, bfull, bounce):
                kpad, f_out = _layer_dims(li)
                fo = f_out

                # --- b4 = Z4, straight to bounce + SBUF state ---
                b4 = bpool.tile([128, TPC, HID], dt.bfloat16, tag="bst")
                for t in range(TPC):
                    zp = zpool.tile([128, HID], dt.float32, tag="z")
                    dense_tile(zp, hT_in, li, 4, t, False)
                    nc.vector.tensor_copy(b4[:, t, :fo], zp[:, :fo])
                    nc.sync.dma_start(
                        bounce.ap()[t * 128 : (t + 1) * 128, :], b4[:, t, :fo]
                    )
                do_exchange(bounce, bfull)

                b_prev2 = None  # b_{k+2}
                b_prev1 = b4  # b_{k+1} (already in bfull)
                for kth in (3, 2, 1, 0):
                    is_final = kth == 0
                    b_new = (
                        None
                        if is_final
                        else bpool.tile([128, TPC, HID], dt.bfloat16, tag="bst")
                    )
                    X = None
                    for t in range(TPC):
                        if no_gather:
                            xw = 1
                            X = xpool.tile([128, xw, fo], dt.bfloat16, tag="X")
                            nc.gpsimd.dma_gather(
                                X[:],
                                bfull.ap(),
                                idx_sb[:, t * cpt * 8 : t * cpt * 8 + 8],
                                xw * CHUNK,
                                xw * CHUNK,
                                fo,
                                single_packet=False,
                                queue_num=gather_q(),
                            )
                            xoff = 0
                        else:
                            if t % GGRP == 0:
                                g = t // GGRP
                                gw = GGRP * cpt
                                X = xpool.tile([128, gw, fo], dt.bfloat16, tag="X")
                                nc.gpsimd.dma_gather(
                                    X[:],
                                    bfull.ap(),
                                    idx_sb[:, g * gw * 8 : (g + 1) * gw * 8],
                                    gw * CHUNK,
                                    gw * CHUNK,
                                    fo,
                                    single_packet=False,
                                    queue_num=gather_q(),
                                )
                            xoff = (t % GGRP) * cpt
                        rp = rpool.tile([128, HID], dt.float32, tag="r")
                        for j in range(cpt):
                            e0 = (t * cpt + j) * CHUNK
                            nc.tensor.matmul(
                                rp[:, :fo],
                                sblk_sb[:, e0 : e0 + CHUNK],
                                X[:, xoff + (j if not no_gather else 0), :],
                                start=(j == 0),
                                stop=(j == cpt - 1),
                            )
                        zp = zpool.tile([128, HID], dt.float32, tag="z")
                        dense_tile(zp, hT_in, li, kth, t, is_final)
                        z_sb = spool.tile([128, HID], dt.bfloat16, tag="ztmp")
                        nc.vector.tensor_copy(z_sb[:, :fo], zp[:, :fo])
                        if kth == 3:
                            nc.vector.tensor_add(
                                b_new[:, t, :fo], z_sb[:, :fo], rp[:, :fo]
                            )
                        elif not is_final:
                            tmp = spool.tile([128, HID], dt.bfloat16, tag="ttmp")
                            nc.vector.tensor_add(tmp[:, :fo], z_sb[:, :fo], rp[:, :fo])
                            nc.vector.tensor_sub(
                                b_new[:, t, :fo], tmp[:, :fo], b_prev2[:, t, :fo]
                            )
                        else:
                            # out = relu(Z0L + 0.5*P2 - b2)
                            a1 = spool.tile([128, HID], dt.float32, tag="a1")
                            nc.vector.tensor_scalar_mul(a1[:, :fo], rp[:, :fo], 0.5)
                            a2 = spool.tile([128, HID], dt.bfloat16, tag="ttmp")
                            nc.vector.tensor_sub(
                                a2[:, :fo], z_sb[:, :fo], b_prev2[:, t, :fo]
                            )
                            if li < 2:
                                h = spool.tile([128, HID], dt.bfloat16, tag="h")
                                nc.vector.tensor_add(h[:, :fo], a1[:, :fo], a2[:, :fo])
                                nc.vector.tensor_relu(h[:, :fo], h[:, :fo])
                                # transpose into hT_out
                                for c2 in range(fo // 128):
                                    tp = tpool.tile([128, 128], dt.bfloat16, tag="tp")
                                    nc.tensor.transpose(
                                        tp[:],
                                        h[:, c2 * 128 : (c2 + 1) * 128],
                                        ident_sb[:],
                                    )
                                    nc.vector.tensor_copy(
                                        hT_out[:, c2, t * 128 : (t + 1) * 128], tp[:]
                                    )
                            else:
                                hf = spool.tile([128, F2PAD], dt.float32, tag="hf")
                                nc.vector.tensor_add(hf[:], a1[:, :fo], a2[:, :fo])
                                nc.vector.tensor_relu(hf[:], hf[:])
                                nc.sync.dma_start(
                                    out_d.ap()[t * 128 : (t + 1) * 128, :], hf[:]
                                )
                        if not is_final:
                            nc.sync.dma_start(
                                bounce.ap()[t * 128 : (t + 1) * 128, :],
                                b_new[:, t, :fo],
                            )
                    if not is_final:
                        do_exchange(bounce, bfull)
                        b_prev2 = b_prev1
                        b_prev1 = b_new

            for _ in range(reps):
                hT1 = hpool.tile([128, HID // 128, NROWS], dt.bfloat16, tag="hTn")
                run_layer(0, hT0, hT1, bfull_a, bounce_a)
                hT2 = hpool.tile([128, HID // 128, NROWS], dt.bfloat16, tag="hTn")
                run_layer(1, hT1, hT2, bfull_a, bounce_a)
                run_layer(2, hT2, None, bfull_b, bounce_b)

    nc.compile()
    return nc


# ----------------------------------------------------------------- runner ---

_CACHE = {}


def _get_nc(cpt, reps=1, **variant):
    key = (cpt, reps, tuple(sorted(variant.items())))
    if key not in _CACHE:
        _CACHE[key] = build_bass(cpt, reps, **variant)
    return _CACHE[key]


def make_in_maps(inputs):
    x = np.asarray(inputs["x"], np.float32)
    edge_index = np.asarray(inputs["edge_index"])
    meta, cores = _build_all(edge_index)
    gslot, slot_node = meta["gslot"], meta["slot_node"]
    cpt, epad = meta["cpt"], meta["epad"]

    weights = _fuse_weights(inputs)
    ident = np.eye(128, dtype=BF16)

    x_slot = np.zeros((NCORES * NROWS, KPAD_IN), np.float32)
    x_slot[gslot, :F_IN] = x

    in_maps = []
    for c in range(NCORES):
        idx, sval, dloc = cores[c]
        m = {
            "xT": np.ascontiguousarray(
                x_slot[c * NROWS : (c + 1) * NROWS].T
            ).astype(BF16),
            "idx": _pack_idx(idx, GGRP * cpt * CHUNK),
            "sblk": _build_sblocks(sval, dloc, epad),
            "ident": ident,
        }
        for li in range(3):
            wf, wrest, bias = weights[li]
            m[f"wf{li}"] = wf
            m[f"wr{li}"] = wrest
            m[f"bias{li}"] = bias
        in_maps.append(m)
    return in_maps, meta


def assemble_output(results, meta):
    slot_node = meta["slot_node"]
    out_slot = np.concatenate([r["out_sh"] for r in results], axis=0)
    out = np.zeros((N, BOT), np.float32)
    valid = slot_node >= 0
    out[slot_node[valid]] = out_slot[valid][:, :BOT]
    return out


def kernel(**inputs):
    from concourse import bass_utils

    in_maps, meta = make_in_maps(inputs)
    nc = _get_nc(meta["cpt"])
    res = bass_utils.run_bass_kernel_spmd(nc, in_maps, core_ids=list(range(NCORES)))
    return assemble_output(res.results, meta)

